# revision 52
# baseline (speedup 1.0000x reference)
"""nn_Attention4DDownsample — Trainium2 Bass/Tile kernel (v3).

Sharding: pure data parallel over batch (8 cores x 32 examples), weights
replicated.  All matmuls bf16.

Key structure (vs the v1 baseline):
- No im2col: the 3x3-s2 depthwise taps read strided 7x7 windows of the flat
  [c, e, 14, 14] tensors directly (4D access patterns on the matmul rhs).
  Border taps use shrunken valid regions; a full-coverage matmul (centre tap
  / bias matmul) opens each PSUM accumulation group.
- v_local is computed as 9 *diagonal* matmuls on v (N-bound), not as dense
  384->512 taps from x (3x less PE work).  The v_local bias (plus the
  attention-value bias vb, which folds in because softmax rows sum to 1)
  enters via a K=49 identity-tile matmul that opens the PSUM group.
- Attention is software-pipelined across example groups: logits+exp for
  group g overlap denominators+AV for group g-1; softmax normalisation is
  applied to the AV *output* (oe), so AV needs only the exponentials.
- The projection is interleaved into the attention pipeline per 8-example
  block, keeping the loop tail short.
- One global PSUM pool with 4 shared tags (uniform 1-bank tiles + views):
  cross-iteration bank reuse pairs phase-1 tags with the attention tiles
  that drain earliest, so back-to-back iterations overlap.
- Weights/constants are DMA'd and the block-diag q buffer zeroed once,
  outside the benchmark loop.
"""

import contextlib
import os
import sys

import numpy as np

for _p in ("/opt/trn_rl_repo", "/root/.axon_site/_ro/trn_rl_repo"):
    if _p not in sys.path and os.path.isdir(_p):
        sys.path.append(_p)

import ml_dtypes  # noqa: E402

import concourse.bacc as bacc  # noqa: E402
import concourse.mybir as mybir  # noqa: E402
import concourse.tile as tile  # noqa: E402

BF16 = mybir.dt.bfloat16
F32 = mybir.dt.float32
AF = mybir.ActivationFunctionType

B, DIM, RES = 256, 384, 14
H, KD, D = 8, 16, 64
NH_KD, DH = H * KD, H * D  # 128, 512
OUT_DIM = 768
RES2 = 7
N, N2 = RES * RES, RES2 * RES2  # 196, 49
SCALE = KD ** -0.5
NCORES = 8
E = B // NCORES  # 32 examples per core

KC = DIM // 128        # 3 contraction chunks of x-channels
VMC = DH // 128        # 4 m-chunks of v channels
PMC = OUT_DIM // 128   # 6 m-chunks of proj out channels
HQ = H * N2            # 392
K0, K1 = 128, N - 128  # key split 0:128 / 128:196

NEG = 8   # examples per block
GEX = 2   # examples per attention group

# tap order: centre (full coverage, opens the PSUM group) first
TAPS = [4, 0, 1, 2, 3, 5, 6, 7, 8]


def _tap_geom(d):
    """(out_lo, valid, half_idx, sub_lo) for a 1-D tap offset d in {0,1,2}.

    Output coord i reads input r = 2i + d - 1 on the 14-grid, which
    factorises as r = 2*ri + rr.
    """
    if d == 0:
        return 1, 6, 1, 0   # r = 2(i-1)+1, valid i in 1..6
    if d == 1:
        return 0, 7, 0, 0   # r = 2i
    return 0, 7, 1, 0       # r = 2i+1


def _bf(x):
    return np.ascontiguousarray(np.asarray(x, np.float32).astype(ml_dtypes.bfloat16))


def _f32(x):
    return np.ascontiguousarray(np.asarray(x, np.float32))


def host_prep(inputs):
    """Fold BN, build fused q taps, diag v_local taps, bias tables."""
    ii = {k: (np.asarray(v) if np.asarray(v).dtype == np.int32
              else np.asarray(v, np.float32)) for k, v in inputs.items()}

    kw = ii["k_w"][:, :, 0, 0] * ii["k_bn_s"][:, None]          # [128,384]
    kb = ii["k_b"] * ii["k_bn_s"] + ii["k_bn_b"]                # [128]
    qw = SCALE * ii["q_bn_s"][:, None] * ii["q_proj_w"][:, :, 0, 0]   # [128,384]
    qlw = ii["q_local_w"][:, 0].reshape(DIM, 9).copy()                # [384,9]
    qlw[:, 4] += 1.0                                                  # pool
    q_taps = np.einsum("md,dt->tdm", qw, qlw)                         # [9,384,128]
    qb = (SCALE * (ii["q_bn_s"] * ii["q_proj_b"] + ii["q_bn_b"])
          + qw @ ii["q_local_b"])                                     # [128]
    vw = ii["v_w"][:, :, 0, 0] * ii["v_bn_s"][:, None]          # [512,384]
    vb = ii["v_b"] * ii["v_bn_s"] + ii["v_bn_b"]                # [512]
    vlw = ii["vl_w"][:, 0].reshape(DH, 9) * ii["vl_bn_s"][:, None]  # [512,9]
    vlb = ii["vl_b"] * ii["vl_bn_s"] + ii["vl_bn_b"]            # [512]
    vd = np.zeros((VMC, 9, 128, 128), np.float32)
    for m in range(VMC):
        for t in range(9):
            np.fill_diagonal(vd[m, t], vlw[128 * m:128 * (m + 1), t])
    # v_local bias: vlb + vb reaching interior taps + vb (softmax-sum fold)
    interior = np.zeros((9, N2), np.float32)
    for t in range(9):
        di, dj = t // 3, t % 3
        for i in range(RES2):
            for j in range(RES2):
                r, c = 2 * i + di - 1, 2 * j + dj - 1
                if 0 <= r < RES and 0 <= c < RES:
                    interior[t, i * RES2 + j] = 1.0
    blb = vlb[:, None] + vb[:, None] * (vlw @ interior) + vb[:, None]  # [512,49]
    blbT = np.transpose(blb.reshape(VMC, 128, N2), (2, 0, 1))          # [49,4,128]
    i49 = np.ascontiguousarray(
        np.eye(N2, dtype=np.float32)[:, None, :].repeat(NEG, 1)
        .reshape(N2, NEG * N2))
    bias = ii["attn_biases"][:, ii["bias_idxs"]]                # [8,49,196]
    biasT = np.transpose(bias, (2, 0, 1)).reshape(N, HQ)        # [196,392]
    bt = np.zeros((128, 2, HQ), np.float32)
    bt[:, 0, :] = biasT[0:K0]
    bt[0:K1, 1, :] = biasT[K0:N]
    pw = ii["proj_w"][:, :, 0, 0] * ii["proj_bn_s"][:, None]    # [768,512]
    pb = ii["proj_b"] * ii["proj_bn_s"] + ii["proj_bn_b"]       # [768]

    shared = {
        "kw_t": _bf(kw.T),                        # [384,128]
        "kb": _f32(kb)[:, None],                  # [128,1]
        "q_taps": _bf(q_taps),                    # [9,384,128]
        "qb": _f32(qb)[:, None],                  # [128,1]
        "vw_t": _bf(vw.T),                        # [384,512]
        "vd": _bf(vd),                            # [4,9,128,128]
        "blbT": _bf(blbT),                        # [49,4,128]
        "i49": _bf(i49),                          # [49,392]
        "bt": _bf(bt),                            # [128,2,392]
        "ident": _bf(np.eye(128)),                # [128,128]
        "pw_t": _bf(pw.T),                        # [512,768]
        "pb": _f32(np.ascontiguousarray(pb.reshape(PMC, 128).T)),  # [128,6]
        "ones_t": _bf(np.ones((128, 128))),       # [128,128]
    }

    x = ii["x"].reshape(NCORES, E, DIM, RES, RES)
    x_shards = []
    for c in range(NCORES):
        xflat = _bf(np.transpose(x[c], (1, 0, 2, 3)).reshape(DIM, E * N))
        x_shards.append({"xfl": xflat})
    return shared, x_shards


def build_nc(e=E, loops=0, parts=15):
    """Build the Bass program for one core with `e` examples.

    loops>0 wraps the per-iteration work in a hardware For loop; weight
    loads and the qbd memset stay outside it.
    parts: bitmask 1=phase1 4=attention 8=proj (bench only).
    """
    nc = bacc.Bacc("TRN2", target_bir_lowering=False, debug=False,
                   enable_asserts=False, num_devices=NCORES)

    def din(name, shape, dtype=BF16):
        return nc.dram_tensor(name, list(shape), dtype, kind="ExternalInput").ap()

    dd = {
        "xfl_d": din("xfl", (DIM, e * N)),
        "kw_d": din("kw_t", (DIM, 128)),
        "kb_d": din("kb", (128, 1), F32),
        "qt_d": din("q_taps", (9, DIM, 128)),
        "qb_d": din("qb", (128, 1), F32),
        "vw_d": din("vw_t", (DIM, DH)),
        "vd_d": din("vd", (VMC, 9, 128, 128)),
        "blbT_d": din("blbT", (N2, VMC, 128)),
        "i49_d": din("i49", (N2, NEG * N2)),
        "bt_d": din("bt", (128, 2, HQ)),
        "id_d": din("ident", (128, 128)),
        "pw_d": din("pw_t", (DH, OUT_DIM)),
        "pb_d": din("pb", (128, PMC), F32),
        "on_d": din("ones_t", (128, 128)),
        "out_d": nc.dram_tensor("out", [e, OUT_DIM, N2], F32,
                                kind="ExternalOutput").ap(),
    }

    with tile.TileContext(nc) as tc:
        build_body(nc, tc, e, dd, parts, loops)

    nc.compile()
    return nc


def build_body(nc, tc, e, dd, parts=15, loops=0):
    ndw = e // NEG
    ngrp = e // GEX
    assert e % NEG == 0 and e % GEX == 0

    with tc.tile_pool(name="const", bufs=1) as cp:
        kw_sb = cp.tile([128, KC, 128], BF16, tag="kw")
        kb_sb = cp.tile([128, 1], F32, tag="kb")
        qt_sb = cp.tile([128, KC, 9, 128], BF16, tag="qt")
        qb_sb = cp.tile([128, 1], F32, tag="qb")
        vw_sb = cp.tile([128, KC, DH], BF16, tag="vw")
        vd_sb = cp.tile([128, VMC, 9, 128], BF16, tag="vd")
        blbT_sb = cp.tile([N2, VMC, 128], BF16, tag="blbT")
        i49_sb = cp.tile([N2, NEG * N2], BF16, tag="i49")
        bt_sb = cp.tile([128, 2, HQ], BF16, tag="bt")
        id_sb = cp.tile([128, 128], BF16, tag="id")
        on_sb = cp.tile([128, 128], BF16, tag="on")
        pw_sb = cp.tile([128, VMC, OUT_DIM], BF16, tag="pw")
        pb_sb = cp.tile([128, PMC], F32, tag="pb")

        for c in range(KC):
            nc.scalar.dma_start(out=kw_sb[:, c, :],
                                in_=dd["kw_d"][128 * c:128 * (c + 1), :])
            nc.scalar.dma_start(out=vw_sb[:, c, :],
                                in_=dd["vw_d"][128 * c:128 * (c + 1), :])
            nc.scalar.dma_start(out=qt_sb[:, c, :, :],
                                in_=dd["qt_d"][:, 128 * c:128 * (c + 1), :]
                                .rearrange("t p m -> p t m"))
        nc.scalar.dma_start(out=kb_sb, in_=dd["kb_d"])
        nc.scalar.dma_start(out=qb_sb, in_=dd["qb_d"])
        nc.scalar.dma_start(out=vd_sb,
                            in_=dd["vd_d"].rearrange("m t p c -> p m t c"))
        nc.scalar.dma_start(out=blbT_sb, in_=dd["blbT_d"])
        nc.scalar.dma_start(out=i49_sb, in_=dd["i49_d"])
        nc.scalar.dma_start(out=bt_sb, in_=dd["bt_d"])
        nc.scalar.dma_start(out=id_sb, in_=dd["id_d"])
        nc.scalar.dma_start(out=on_sb, in_=dd["on_d"])
        for m in range(VMC):
            nc.scalar.dma_start(out=pw_sb[:, m, :],
                                in_=dd["pw_d"][128 * m:128 * (m + 1), :])
        nc.scalar.dma_start(out=pb_sb, in_=dd["pb_d"])

        with (tc.tile_pool(name="persist", bufs=1) as pp,
              tc.tile_pool(name="stream", bufs=2) as st,
              tc.tile_pool(name="psum", bufs=2, space="PSUM") as ps):
            k_s = pp.tile([128, e, N], BF16, tag="k_s")
            q_s = pp.tile([128, e, N2], BF16, tag="q_s")
            qbd = pp.tile([128, e, HQ], BF16, tag="qbd")
            vt0 = pp.tile([128, e, DH], BF16, tag="vt0")
            vt1 = pp.tile([128, e, DH], BF16, tag="vt1")
            vl_sb = pp.tile([128, VMC, e, N2], BF16, tag="vl_sb")

            nc.gpsimd.memset(qbd, 0)

            consts = dict(kw_sb=kw_sb, kb_sb=kb_sb, qt_sb=qt_sb, qb_sb=qb_sb,
                          vw_sb=vw_sb, vd_sb=vd_sb, blbT_sb=blbT_sb,
                          i49_sb=i49_sb, bt_sb=bt_sb, id_sb=id_sb,
                          on_sb=on_sb, pw_sb=pw_sb, pb_sb=pb_sb)
            tiles = dict(k_s=k_s, q_s=q_s, qbd=qbd, vt0=vt0, vt1=vt1,
                         vl_sb=vl_sb)
            with (tc.For_i(0, loops, 1) if loops else contextlib.nullcontext()):
                if parts & 1:
                    phase1(nc, tc, e, ndw, dd, st, ps, consts, tiles)
                if parts & 4:
                    phase23(nc, tc, e, ngrp, ndw, dd, st, ps, consts, tiles,
                            parts)


def _psum(ps, tag):
    """Uniform 1-bank PSUM tile; callers take views."""
    return ps.tile([128, DH], F32, tag=tag, name=tag)


def phase1(nc, tc, e, ndw, dd, st, ps, c, t):
    """k, v (natural), vT (direct), q (fused taps), v_local (diag taps)."""
    xfl_r = dd["xfl_d"].rearrange("(c p) (ee q) -> p c ee q", p=128, q=N)
    k_s, q_s = t["k_s"], t["q_s"]
    vt0, vt1, vl_sb = t["vt0"], t["vt1"], t["vl_sb"]
    for blk in range(ndw):
        e0 = blk * NEG
        xf = st.tile([128, KC, NEG, N], BF16, tag="xf")
        for kc in range(KC):
            nc.sync.dma_start(out=xf[:, kc, :, :],
                              in_=xfl_r[:, kc, e0:e0 + NEG, :])
        xfv = xf.rearrange("p c ee (ri rr ci cc) -> p c ee ri rr ci cc",
                           ri=RES2, rr=2, ci=RES2, cc=2)
        vs = st.tile([128, VMC, NEG, N], BF16, tag="vs")

        # ---- k (natural layout), pairs of examples ----
        for ee in range(0, NEG, 2):
            kp = _psum(ps, "t0")[:, 0:2 * N].rearrange("p (a b) -> p a b", a=2)
            for kc in range(KC):
                nc.tensor.matmul(kp, c["kw_sb"][:, kc, :],
                                 xf[:, kc, ee:ee + 2, :],
                                 start=(kc == 0), stop=(kc == KC - 1))
            nc.scalar.activation(out=k_s[:, e0 + ee:e0 + ee + 2, :],
                                 in_=kp, func=AF.Identity, bias=c["kb_sb"])

        # ---- v natural, pairs x m-chunks; drains spread over engines ----
        for ee in range(0, NEG, 2):
            for m in range(VMC):
                vp = _psum(ps, "t0")[:, 0:2 * N].rearrange("p (a b) -> p a b",
                                                           a=2)
                for kc in range(KC):
                    nc.tensor.matmul(vp,
                                     c["vw_sb"][:, kc, 128 * m:128 * (m + 1)],
                                     xf[:, kc, ee:ee + 2, :],
                                     start=(kc == 0), stop=(kc == KC - 1))
                dst = vs[:, m, ee:ee + 2, :]
                if m < 2:
                    nc.vector.tensor_copy(dst, vp)
                else:
                    nc.scalar.activation(out=dst, in_=vp, func=AF.Copy)

        # ---- vT direct from x (keys on partitions) ----
        for ee in range(NEG):
            ex = e0 + ee
            v0 = _psum(ps, "t1")
            v1 = _psum(ps, "t2")[0:K1, :]
            for kc in range(KC):
                nc.tensor.matmul(v0, xf[:, kc, ee, 0:K0], c["vw_sb"][:, kc, :],
                                 start=(kc == 0), stop=(kc == KC - 1))
            for kc in range(KC):
                nc.tensor.matmul(v1, xf[:, kc, ee, K0:N], c["vw_sb"][:, kc, :],
                                 start=(kc == 0), stop=(kc == KC - 1))
            if ee % 2 == 0:
                nc.scalar.activation(out=vt0[:, ex, :], in_=v0, func=AF.Copy)
                nc.vector.tensor_copy(vt1[0:K1, ex, :], v1)
            else:
                nc.vector.tensor_copy(vt0[:, ex, :], v0)
                nc.scalar.activation(out=vt1[0:K1, ex, :], in_=v1,
                                     func=AF.Copy)

        # ---- q fused taps (centre first opens the PSUM group) ----
        qp = _psum(ps, "t3")[:, 0:NEG * N2].rearrange(
            "p (ee a b) -> p ee a b", ee=NEG, a=RES2, b=RES2)
        mms = []
        for ti, tp in enumerate(TAPS):
            di, dj = tp // 3, tp % 3
            io, vi, rr, ri0 = _tap_geom(di)
            jo, vj, cc, ci0 = _tap_geom(dj)
            for kc in range(KC):
                first = ti == 0 and kc == 0
                last = ti == 8 and kc == KC - 1
                mm = nc.tensor.matmul(
                    qp[:, :, io:io + vi, jo:jo + vj],
                    c["qt_sb"][:, kc, tp, :],
                    xfv[:, kc, :, ri0:ri0 + vi, rr, ci0:ci0 + vj, cc],
                    start=first, stop=last,
                    skip_group_check=not (first or last))
                mms.append(mm)
        _order_group(mms)
        nc.scalar.activation(out=q_s[:, e0:e0 + NEG, :],
                             in_=qp.rearrange("p ee a b -> p ee (a b)"),
                             func=AF.Identity, bias=c["qb_sb"])
        # blockdiag q for this block: scatter via sbuf->sbuf DMA (scalar
        # queue — must not block the sync queue's xf prefetch)
        for h in range(H):
            nc.scalar.dma_start(
                out=t["qbd"][16 * h:16 * (h + 1), e0:e0 + NEG,
                             N2 * h:N2 * (h + 1)],
                in_=q_s[16 * h:16 * (h + 1), e0:e0 + NEG, :])

        # ---- v_local: bias matmul + diag taps on v ----
        vsv = vs.rearrange("p m ee (ri rr ci cc) -> p m ee ri rr ci cc",
                           ri=RES2, rr=2, ci=RES2, cc=2)
        for m in range(VMC):
            dp = _psum(ps, "t3")[:, 0:NEG * N2].rearrange(
                "p (ee a b) -> p ee a b", ee=NEG, a=RES2, b=RES2)
            mms = [nc.tensor.matmul(dp, c["blbT_sb"][:, m, :], c["i49_sb"],
                                    start=True, stop=False)]
            for ti, tp in enumerate(TAPS):
                di, dj = tp // 3, tp % 3
                io, vi, rr, ri0 = _tap_geom(di)
                jo, vj, cc, ci0 = _tap_geom(dj)
                last = ti == 8
                mm = nc.tensor.matmul(
                    dp[:, :, io:io + vi, jo:jo + vj],
                    c["vd_sb"][:, m, tp, :],
                    vsv[:, m, :, ri0:ri0 + vi, rr, ci0:ci0 + vj, cc],
                    start=False, stop=last,
                    skip_group_check=not last)
                mms.append(mm)
            _order_group(mms)
            nc.scalar.activation(
                out=vl_sb[:, m, e0:e0 + NEG, :],
                in_=dp.rearrange("p ee a b -> p ee (a b)"), func=AF.Copy)


def _order_group(mms):
    for mm in mms[1:-1]:
        tile.add_dep_helper(mm.ins, mms[0].ins, sync=False,
                            reason="psum group start first")
        tile.add_dep_helper(mms[-1].ins, mm.ins, sync=False,
                            reason="psum group stop last")
    tile.add_dep_helper(mms[-1].ins, mms[0].ins, sync=False,
                        reason="psum group order")


def phase23(nc, tc, e, ngrp, ndw, dd, st, ps, c, t, parts):
    """Attention (software-pipelined) with the projection interleaved.

    Stage A(g):   PE bias+logits, ACT exp           -> a0/a1 (unnormalised)
    Stage B(g-1): PE denominators + AV, DVE recip   -> oe (unnormalised)
    Stage C(g-1): DVE normalise oe, merge into vl_sb
    After both groups of a block have merged: ACT gelu, PE proj, store.
    """
    k_s, qbd, vt0, vt1, vl_sb = (t["k_s"], t["qbd"], t["vt0"], t["vt1"],
                                 t["vl_sb"])
    live = {}

    def stage_a(grp):
        a0 = st.tile([128, GEX, HQ], BF16, tag="a0", bufs=4)
        a1 = st.tile([128, GEX, HQ], BF16, tag="a1", bufs=4)
        live[grp] = (a0, a1)
        for eg in range(GEX):
            ex = grp * GEX + eg
            l0 = _psum(ps, "t0")[:, 0:HQ]
            l1 = _psum(ps, "t1")[0:K1, 0:HQ]
            nc.tensor.matmul(l0, c["id_sb"], c["bt_sb"][:, 0, :],
                             start=True, stop=False)
            nc.tensor.matmul(l0, k_s[:, ex, 0:K0], qbd[:, ex, :],
                             start=False, stop=True)
            nc.tensor.matmul(l1, c["id_sb"][0:K1, 0:K1], c["bt_sb"][0:K1, 1, :],
                             start=True, stop=False)
            nc.tensor.matmul(l1, k_s[:, ex, K0:N], qbd[:, ex, :],
                             start=False, stop=True)
            nc.scalar.activation(out=a0[:, eg, :], in_=l0, func=AF.Exp)
            nc.scalar.activation(out=a1[0:K1, eg, :], in_=l1, func=AF.Exp)

    def stage_bc(grp):
        a0, a1 = live.pop(grp)
        for eg in range(GEX):
            ex = grp * GEX + eg
            sp = _psum(ps, "t2")[:, 0:HQ]
            nc.tensor.matmul(sp, c["on_sb"], a0[:, eg, :],
                             start=True, stop=False)
            nc.tensor.matmul(sp, c["on_sb"][0:K1, :], a1[0:K1, eg, :],
                             start=False, stop=True)
            rsf = st.tile([128, HQ], F32, tag="rsf")
            nc.vector.reciprocal_approx_fast(out=rsf, in_=sp)
            oe = _psum(ps, "t3")[:, 0:VMC * 2 * N2].rearrange(
                "p (m k q) -> p m k q", m=VMC, k=2, q=N2)
            for m in range(VMC):
                nc.tensor.matmul(oe[:, m, :, :],
                                 vt0[:, ex, 128 * m:128 * (m + 1)],
                                 a0[:, eg, 98 * m:98 * (m + 1)],
                                 start=True, stop=False)
                nc.tensor.matmul(oe[:, m, :, :],
                                 vt1[0:K1, ex, 128 * m:128 * (m + 1)],
                                 a1[0:K1, eg, 98 * m:98 * (m + 1)],
                                 start=False, stop=True)
            rv = rsf.rearrange("p (m k q) -> p m k q", m=VMC, k=2, q=N2)
            nc.vector.tensor_mul(oe[0:64, :, 0, :], oe[0:64, :, 0, :],
                                 rv[0:64, :, 0, :])
            nc.vector.tensor_mul(oe[64:128, :, 1, :], oe[64:128, :, 1, :],
                                 rv[64:128, :, 1, :])
            nc.vector.tensor_add(vl_sb[0:64, :, ex, :],
                                 oe[0:64, :, 0, :],
                                 vl_sb[0:64, :, ex, :])
            nc.vector.tensor_add(vl_sb[64:128, :, ex, :],
                                 oe[64:128, :, 1, :],
                                 vl_sb[64:128, :, ex, :])

    def proj_blk(blk):
        e0 = blk * NEG
        nc.scalar.activation(out=vl_sb[:, :, e0:e0 + NEG, :],
                             in_=vl_sb[:, :, e0:e0 + NEG, :], func=AF.Gelu)
        if not parts & 8:
            return
        for m in range(PMC):
            pj = _psum(ps, "t3")[:, 0:NEG * N2].rearrange(
                "p (ee q) -> p ee q", ee=NEG)
            for kc in range(VMC):
                nc.tensor.matmul(pj, c["pw_sb"][:, kc, 128 * m:128 * (m + 1)],
                                 vl_sb[:, kc, e0:e0 + NEG, :],
                                 start=(kc == 0), stop=(kc == VMC - 1))
            ost = st.tile([128, NEG, N2], F32, tag="ost")
            nc.vector.tensor_scalar_add(ost, pj, c["pb_sb"][:, m:m + 1])
            nc.scalar.dma_start(
                out=dd["out_d"][e0:e0 + NEG, 128 * m:128 * (m + 1), :]
                .rearrange("e p q -> p e q"),
                in_=ost)

    gpb = NEG // GEX  # groups per block

    def maybe_proj(done_grp):
        if done_grp >= 0 and (done_grp + 1) % gpb == 0:
            proj_blk((done_grp + 1) // gpb - 1)

    OFF = 3  # pipeline depth in groups (stage_a leads stage_bc)
    for grp in range(OFF):
        stage_a(grp)
    for grp in range(OFF, ngrp):
        stage_a(grp)
        stage_bc(grp - OFF)
        maybe_proj(grp - OFF)
    for grp in range(ngrp - OFF, ngrp):
        stage_bc(grp)
        maybe_proj(grp)


_CACHE = {}


def _get_nc(e=E, loops=0, parts=15):
    key = (e, loops, parts)
    if key not in _CACHE:
        _CACHE[key] = build_nc(e, loops, parts)
    return _CACHE[key]


def kernel(**inputs):
    from concourse.bass_utils import run_bass_kernel_spmd

    shared, x_shards = host_prep(inputs)
    nc = _get_nc(E)
    in_maps = [{**shared, **x_shards[c]} for c in range(NCORES)]
    res = run_bass_kernel_spmd(nc, in_maps, core_ids=list(range(NCORES)))
    out = np.concatenate([r["out"] for r in res.results], axis=0)
    return out.reshape(B, OUT_DIM, RES2, RES2).astype(np.float32)


# revision 57
# speedup vs baseline: 1.1269x; 1.1269x over previous
"""nn_Attention4DDownsample — Trainium2 Bass/Tile kernel (v3).

Sharding: pure data parallel over batch (8 cores x 32 examples), weights
replicated.  All matmuls bf16.

Key structure (vs the v1 baseline):
- No im2col: the 3x3-s2 depthwise taps read strided 7x7 windows of the flat
  [c, e, 14, 14] tensors directly (4D access patterns on the matmul rhs).
  Border taps use shrunken valid regions; a full-coverage matmul (centre tap
  / bias matmul) opens each PSUM accumulation group.
- v_local is computed as 9 *diagonal* matmuls on v (N-bound), not as dense
  384->512 taps from x (3x less PE work).  The v_local bias (plus the
  attention-value bias vb, which folds in because softmax rows sum to 1)
  enters via a K=49 identity-tile matmul that opens the PSUM group.
- Attention is software-pipelined across example groups: logits+exp for
  group g overlap denominators+AV for group g-1; softmax normalisation is
  applied to the AV *output* (oe), so AV needs only the exponentials.
- The projection is interleaved into the attention pipeline per 8-example
  block, keeping the loop tail short.
- One global PSUM pool with 4 shared tags (uniform 1-bank tiles + views):
  cross-iteration bank reuse pairs phase-1 tags with the attention tiles
  that drain earliest, so back-to-back iterations overlap.
- Weights/constants are DMA'd and the block-diag q buffer zeroed once,
  outside the benchmark loop.
"""

import contextlib
import os
import sys

import numpy as np

for _p in ("/opt/trn_rl_repo", "/root/.axon_site/_ro/trn_rl_repo"):
    if _p not in sys.path and os.path.isdir(_p):
        sys.path.append(_p)

import ml_dtypes  # noqa: E402

import concourse.bacc as bacc  # noqa: E402
import concourse.mybir as mybir  # noqa: E402
import concourse.tile as tile  # noqa: E402

BF16 = mybir.dt.bfloat16
F32 = mybir.dt.float32
AF = mybir.ActivationFunctionType

B, DIM, RES = 256, 384, 14
H, KD, D = 8, 16, 64
NH_KD, DH = H * KD, H * D  # 128, 512
OUT_DIM = 768
RES2 = 7
N, N2 = RES * RES, RES2 * RES2  # 196, 49
SCALE = KD ** -0.5
NCORES = 8
E = B // NCORES  # 32 examples per core

KC = DIM // 128        # 3 contraction chunks of x-channels
VMC = DH // 128        # 4 m-chunks of v channels
PMC = OUT_DIM // 128   # 6 m-chunks of proj out channels
HQ = H * N2            # 392
K0, K1 = 128, N - 128  # key split 0:128 / 128:196

NEG = 8   # examples per block
GEX = 2   # examples per attention group

# Tunables for A/B experiments (read at build time).
OPTS = {
    "off": 3,         # attention pipeline depth (groups)
    "proj_delay": 0,  # extra groups of slack before each proj block
    "abufs": 4,       # a0/a1 rotation depth
}

# tap order: centre (full coverage, opens the PSUM group) first
TAPS = [4, 0, 1, 2, 3, 5, 6, 7, 8]


def _tap_geom(d):
    """(out_lo, valid, half_idx, sub_lo) for a 1-D tap offset d in {0,1,2}.

    Output coord i reads input r = 2i + d - 1 on the 14-grid, which
    factorises as r = 2*ri + rr.
    """
    if d == 0:
        return 1, 6, 1, 0   # r = 2(i-1)+1, valid i in 1..6
    if d == 1:
        return 0, 7, 0, 0   # r = 2i
    return 0, 7, 1, 0       # r = 2i+1


def _bf(x):
    return np.ascontiguousarray(np.asarray(x, np.float32).astype(ml_dtypes.bfloat16))


def _f32(x):
    return np.ascontiguousarray(np.asarray(x, np.float32))


def host_prep(inputs):
    """Fold BN, build fused q taps, diag v_local taps, bias tables."""
    ii = {k: (np.asarray(v) if np.asarray(v).dtype == np.int32
              else np.asarray(v, np.float32)) for k, v in inputs.items()}

    kw = ii["k_w"][:, :, 0, 0] * ii["k_bn_s"][:, None]          # [128,384]
    kb = ii["k_b"] * ii["k_bn_s"] + ii["k_bn_b"]                # [128]
    qw = SCALE * ii["q_bn_s"][:, None] * ii["q_proj_w"][:, :, 0, 0]   # [128,384]
    qlw = ii["q_local_w"][:, 0].reshape(DIM, 9).copy()                # [384,9]
    qlw[:, 4] += 1.0                                                  # pool
    q_taps = np.einsum("md,dt->tdm", qw, qlw)                         # [9,384,128]
    qb = (SCALE * (ii["q_bn_s"] * ii["q_proj_b"] + ii["q_bn_b"])
          + qw @ ii["q_local_b"])                                     # [128]
    vw = ii["v_w"][:, :, 0, 0] * ii["v_bn_s"][:, None]          # [512,384]
    vb = ii["v_b"] * ii["v_bn_s"] + ii["v_bn_b"]                # [512]
    vlw = ii["vl_w"][:, 0].reshape(DH, 9) * ii["vl_bn_s"][:, None]  # [512,9]
    vlb = ii["vl_b"] * ii["vl_bn_s"] + ii["vl_bn_b"]            # [512]
    vd = np.zeros((VMC, 9, 128, 128), np.float32)
    for m in range(VMC):
        for t in range(9):
            np.fill_diagonal(vd[m, t], vlw[128 * m:128 * (m + 1), t])
    # v_local bias: vlb + vb reaching interior taps + vb (softmax-sum fold)
    interior = np.zeros((9, N2), np.float32)
    for t in range(9):
        di, dj = t // 3, t % 3
        for i in range(RES2):
            for j in range(RES2):
                r, c = 2 * i + di - 1, 2 * j + dj - 1
                if 0 <= r < RES and 0 <= c < RES:
                    interior[t, i * RES2 + j] = 1.0
    blb = vlb[:, None] + vb[:, None] * (vlw @ interior) + vb[:, None]  # [512,49]
    blbT = np.transpose(blb.reshape(VMC, 128, N2), (2, 0, 1))          # [49,4,128]
    i49 = np.ascontiguousarray(
        np.eye(N2, dtype=np.float32)[:, None, :].repeat(NEG, 1)
        .reshape(N2, NEG * N2))
    bias = ii["attn_biases"][:, ii["bias_idxs"]]                # [8,49,196]
    biasT = np.transpose(bias, (2, 0, 1)).reshape(N, HQ)        # [196,392]
    bt = np.zeros((128, 2, HQ), np.float32)
    bt[:, 0, :] = biasT[0:K0]
    bt[0:K1, 1, :] = biasT[K0:N]
    pw = ii["proj_w"][:, :, 0, 0] * ii["proj_bn_s"][:, None]    # [768,512]
    pb = ii["proj_b"] * ii["proj_bn_s"] + ii["proj_bn_b"]       # [768]

    shared = {
        "kw_t": _bf(kw.T),                        # [384,128]
        "kb": _f32(kb)[:, None],                  # [128,1]
        "q_taps": _bf(q_taps),                    # [9,384,128]
        "qb": _f32(qb)[:, None],                  # [128,1]
        "vw_t": _bf(vw.T),                        # [384,512]
        "vd": _bf(vd),                            # [4,9,128,128]
        "blbT": _bf(blbT),                        # [49,4,128]
        "i49": _bf(i49),                          # [49,392]
        "bt": _bf(bt),                            # [128,2,392]
        "ident": _bf(np.eye(128)),                # [128,128]
        "pw_t": _bf(pw.T),                        # [512,768]
        "pb": _f32(np.ascontiguousarray(pb.reshape(PMC, 128).T)),  # [128,6]
        "ones_t": _bf(np.ones((128, 128))),       # [128,128]
    }

    x = ii["x"].reshape(NCORES, E, DIM, RES, RES)
    x_shards = []
    for c in range(NCORES):
        xflat = _bf(np.transpose(x[c], (1, 0, 2, 3)).reshape(DIM, E * N))
        x_shards.append({"xfl": xflat})
    return shared, x_shards


def build_nc(e=E, loops=0, parts=15):
    """Build the Bass program for one core with `e` examples.

    loops>0 wraps the per-iteration work in a hardware For loop; weight
    loads and the qbd memset stay outside it.
    parts: bitmask 1=phase1 4=attention 8=proj (bench only).
    """
    nc = bacc.Bacc("TRN2", target_bir_lowering=False, debug=False,
                   enable_asserts=False, num_devices=NCORES)

    def din(name, shape, dtype=BF16):
        return nc.dram_tensor(name, list(shape), dtype, kind="ExternalInput").ap()

    dd = {
        "xfl_d": din("xfl", (DIM, e * N)),
        "kw_d": din("kw_t", (DIM, 128)),
        "kb_d": din("kb", (128, 1), F32),
        "qt_d": din("q_taps", (9, DIM, 128)),
        "qb_d": din("qb", (128, 1), F32),
        "vw_d": din("vw_t", (DIM, DH)),
        "vd_d": din("vd", (VMC, 9, 128, 128)),
        "blbT_d": din("blbT", (N2, VMC, 128)),
        "i49_d": din("i49", (N2, NEG * N2)),
        "bt_d": din("bt", (128, 2, HQ)),
        "id_d": din("ident", (128, 128)),
        "pw_d": din("pw_t", (DH, OUT_DIM)),
        "pb_d": din("pb", (128, PMC), F32),
        "on_d": din("ones_t", (128, 128)),
        "out_d": nc.dram_tensor("out", [e, OUT_DIM, N2], F32,
                                kind="ExternalOutput").ap(),
    }

    with tile.TileContext(nc) as tc:
        build_body(nc, tc, e, dd, parts, loops)

    nc.compile()
    return nc


def build_body(nc, tc, e, dd, parts=15, loops=0):
    ndw = e // NEG
    ngrp = e // GEX
    assert e % NEG == 0 and e % GEX == 0

    with tc.tile_pool(name="const", bufs=1) as cp:
        kw_sb = cp.tile([128, KC, 128], BF16, tag="kw")
        kb_sb = cp.tile([128, 1], F32, tag="kb")
        qt_sb = cp.tile([128, KC, 9, 128], BF16, tag="qt")
        qb_sb = cp.tile([128, 1], F32, tag="qb")
        vw_sb = cp.tile([128, KC, DH], BF16, tag="vw")
        vd_sb = cp.tile([128, VMC, 9, 128], BF16, tag="vd")
        blbT_sb = cp.tile([N2, VMC, 128], BF16, tag="blbT")
        i49_sb = cp.tile([N2, NEG * N2], BF16, tag="i49")
        bt_sb = cp.tile([128, 2, HQ], BF16, tag="bt")
        id_sb = cp.tile([128, 128], BF16, tag="id")
        on_sb = cp.tile([128, 128], BF16, tag="on")
        pw_sb = cp.tile([128, VMC, OUT_DIM], BF16, tag="pw")
        pb_sb = cp.tile([128, PMC], F32, tag="pb")

        for c in range(KC):
            nc.scalar.dma_start(out=kw_sb[:, c, :],
                                in_=dd["kw_d"][128 * c:128 * (c + 1), :])
            nc.scalar.dma_start(out=vw_sb[:, c, :],
                                in_=dd["vw_d"][128 * c:128 * (c + 1), :])
            nc.scalar.dma_start(out=qt_sb[:, c, :, :],
                                in_=dd["qt_d"][:, 128 * c:128 * (c + 1), :]
                                .rearrange("t p m -> p t m"))
        nc.scalar.dma_start(out=kb_sb, in_=dd["kb_d"])
        nc.scalar.dma_start(out=qb_sb, in_=dd["qb_d"])
        nc.scalar.dma_start(out=vd_sb,
                            in_=dd["vd_d"].rearrange("m t p c -> p m t c"))
        nc.scalar.dma_start(out=blbT_sb, in_=dd["blbT_d"])
        nc.scalar.dma_start(out=i49_sb, in_=dd["i49_d"])
        nc.scalar.dma_start(out=bt_sb, in_=dd["bt_d"])
        nc.scalar.dma_start(out=id_sb, in_=dd["id_d"])
        nc.scalar.dma_start(out=on_sb, in_=dd["on_d"])
        for m in range(VMC):
            nc.scalar.dma_start(out=pw_sb[:, m, :],
                                in_=dd["pw_d"][128 * m:128 * (m + 1), :])
        nc.scalar.dma_start(out=pb_sb, in_=dd["pb_d"])

        with (tc.tile_pool(name="persist", bufs=1) as pp,
              tc.tile_pool(name="stream", bufs=2) as st,
              tc.tile_pool(name="psum", bufs=2, space="PSUM") as ps):
            k_s = pp.tile([128, e, N], BF16, tag="k_s")
            q_s = pp.tile([128, e, N2], BF16, tag="q_s")
            qbd = pp.tile([128, e, HQ], BF16, tag="qbd")
            vt0 = pp.tile([128, e, DH], BF16, tag="vt0")
            vt1 = pp.tile([128, e, DH], BF16, tag="vt1")
            vl_sb = pp.tile([128, VMC, e, N2], BF16, tag="vl_sb")

            nc.gpsimd.memset(qbd, 0)

            consts = dict(kw_sb=kw_sb, kb_sb=kb_sb, qt_sb=qt_sb, qb_sb=qb_sb,
                          vw_sb=vw_sb, vd_sb=vd_sb, blbT_sb=blbT_sb,
                          i49_sb=i49_sb, bt_sb=bt_sb, id_sb=id_sb,
                          on_sb=on_sb, pw_sb=pw_sb, pb_sb=pb_sb)
            tiles = dict(k_s=k_s, q_s=q_s, qbd=qbd, vt0=vt0, vt1=vt1,
                         vl_sb=vl_sb)
            with (tc.For_i(0, loops, 1) if loops else contextlib.nullcontext()):
                if parts & 1:
                    phase1(nc, tc, e, ndw, dd, st, ps, consts, tiles)
                if parts & 4:
                    phase23(nc, tc, e, ngrp, ndw, dd, st, ps, consts, tiles,
                            parts)


def _psum(ps, tag):
    """Uniform 1-bank PSUM tile; callers take views."""
    return ps.tile([128, DH], F32, tag=tag, name=tag)


def phase1(nc, tc, e, ndw, dd, st, ps, c, t):
    """k, v (natural), vT (direct), q (fused taps), v_local (diag taps)."""
    xfl_r = dd["xfl_d"].rearrange("(c p) (ee q) -> p c ee q", p=128, q=N)
    k_s, q_s = t["k_s"], t["q_s"]
    vt0, vt1, vl_sb = t["vt0"], t["vt1"], t["vl_sb"]
    for blk in range(ndw):
        e0 = blk * NEG
        xf = st.tile([128, KC, NEG, N], BF16, tag="xf")
        for kc in range(KC):
            nc.sync.dma_start(out=xf[:, kc, :, :],
                              in_=xfl_r[:, kc, e0:e0 + NEG, :])
        xfv = xf.rearrange("p c ee (ri rr ci cc) -> p c ee ri rr ci cc",
                           ri=RES2, rr=2, ci=RES2, cc=2)
        vs = st.tile([128, VMC, NEG, N], BF16, tag="vs")

        # ---- k (natural layout), pairs of examples ----
        for ee in range(0, NEG, 2):
            kp = _psum(ps, "t0")[:, 0:2 * N].rearrange("p (a b) -> p a b", a=2)
            for kc in range(KC):
                nc.tensor.matmul(kp, c["kw_sb"][:, kc, :],
                                 xf[:, kc, ee:ee + 2, :],
                                 start=(kc == 0), stop=(kc == KC - 1))
            nc.scalar.activation(out=k_s[:, e0 + ee:e0 + ee + 2, :],
                                 in_=kp, func=AF.Identity, bias=c["kb_sb"])

        # ---- v natural, pairs x m-chunks; drains spread over engines ----
        for ee in range(0, NEG, 2):
            for m in range(VMC):
                vp = _psum(ps, "t0")[:, 0:2 * N].rearrange("p (a b) -> p a b",
                                                           a=2)
                for kc in range(KC):
                    nc.tensor.matmul(vp,
                                     c["vw_sb"][:, kc, 128 * m:128 * (m + 1)],
                                     xf[:, kc, ee:ee + 2, :],
                                     start=(kc == 0), stop=(kc == KC - 1))
                dst = vs[:, m, ee:ee + 2, :]
                if m < 2:
                    nc.vector.tensor_copy(dst, vp)
                else:
                    nc.scalar.activation(out=dst, in_=vp, func=AF.Copy)

        # ---- vT direct from x (keys on partitions) ----
        for ee in range(NEG):
            ex = e0 + ee
            v0 = _psum(ps, "t1")
            v1 = _psum(ps, "t2")[0:K1, :]
            for kc in range(KC):
                nc.tensor.matmul(v0, xf[:, kc, ee, 0:K0], c["vw_sb"][:, kc, :],
                                 start=(kc == 0), stop=(kc == KC - 1))
            for kc in range(KC):
                nc.tensor.matmul(v1, xf[:, kc, ee, K0:N], c["vw_sb"][:, kc, :],
                                 start=(kc == 0), stop=(kc == KC - 1))
            if ee % 2 == 0:
                nc.scalar.activation(out=vt0[:, ex, :], in_=v0, func=AF.Copy)
                nc.vector.tensor_copy(vt1[0:K1, ex, :], v1)
            else:
                nc.vector.tensor_copy(vt0[:, ex, :], v0)
                nc.scalar.activation(out=vt1[0:K1, ex, :], in_=v1,
                                     func=AF.Copy)

        # ---- q fused taps (centre first opens the PSUM group) ----
        qp = _psum(ps, "t3")[:, 0:NEG * N2].rearrange(
            "p (ee a b) -> p ee a b", ee=NEG, a=RES2, b=RES2)
        mms = []
        for ti, tp in enumerate(TAPS):
            di, dj = tp // 3, tp % 3
            io, vi, rr, ri0 = _tap_geom(di)
            jo, vj, cc, ci0 = _tap_geom(dj)
            for kc in range(KC):
                first = ti == 0 and kc == 0
                last = ti == 8 and kc == KC - 1
                mm = nc.tensor.matmul(
                    qp[:, :, io:io + vi, jo:jo + vj],
                    c["qt_sb"][:, kc, tp, :],
                    xfv[:, kc, :, ri0:ri0 + vi, rr, ci0:ci0 + vj, cc],
                    start=first, stop=last,
                    skip_group_check=not (first or last))
                mms.append(mm)
        _order_group(mms)
        nc.scalar.activation(out=q_s[:, e0:e0 + NEG, :],
                             in_=qp.rearrange("p ee a b -> p ee (a b)"),
                             func=AF.Identity, bias=c["qb_sb"])
        # blockdiag q for this block: scatter via sbuf->sbuf DMA (scalar
        # queue — must not block the sync queue's xf prefetch)
        for h in range(H):
            nc.scalar.dma_start(
                out=t["qbd"][16 * h:16 * (h + 1), e0:e0 + NEG,
                             N2 * h:N2 * (h + 1)],
                in_=q_s[16 * h:16 * (h + 1), e0:e0 + NEG, :])

        # ---- v_local: bias matmul + diag taps on v ----
        vsv = vs.rearrange("p m ee (ri rr ci cc) -> p m ee ri rr ci cc",
                           ri=RES2, rr=2, ci=RES2, cc=2)
        for m in range(VMC):
            dp = _psum(ps, "t3")[:, 0:NEG * N2].rearrange(
                "p (ee a b) -> p ee a b", ee=NEG, a=RES2, b=RES2)
            mms = [nc.tensor.matmul(dp, c["blbT_sb"][:, m, :], c["i49_sb"],
                                    start=True, stop=False)]
            for ti, tp in enumerate(TAPS):
                di, dj = tp // 3, tp % 3
                io, vi, rr, ri0 = _tap_geom(di)
                jo, vj, cc, ci0 = _tap_geom(dj)
                last = ti == 8
                mm = nc.tensor.matmul(
                    dp[:, :, io:io + vi, jo:jo + vj],
                    c["vd_sb"][:, m, tp, :],
                    vsv[:, m, :, ri0:ri0 + vi, rr, ci0:ci0 + vj, cc],
                    start=False, stop=last,
                    skip_group_check=not last)
                mms.append(mm)
            _order_group(mms)
            nc.scalar.activation(
                out=vl_sb[:, m, e0:e0 + NEG, :],
                in_=dp.rearrange("p ee a b -> p ee (a b)"), func=AF.Copy)


def _order_group(mms):
    for mm in mms[1:-1]:
        tile.add_dep_helper(mm.ins, mms[0].ins, sync=False,
                            reason="psum group start first")
        tile.add_dep_helper(mms[-1].ins, mm.ins, sync=False,
                            reason="psum group stop last")
    tile.add_dep_helper(mms[-1].ins, mms[0].ins, sync=False,
                        reason="psum group order")


def phase23(nc, tc, e, ngrp, ndw, dd, st, ps, c, t, parts):
    """Attention (software-pipelined) with the projection interleaved.

    Stage A(g):   PE bias+logits, ACT exp           -> a0/a1 (unnormalised)
    Stage B(g-1): PE denominators + AV, DVE recip   -> oe (unnormalised)
    Stage C(g-1): DVE normalise oe, merge into vl_sb
    After both groups of a block have merged: ACT gelu, PE proj, store.
    """
    k_s, qbd, vt0, vt1, vl_sb = (t["k_s"], t["qbd"], t["vt0"], t["vt1"],
                                 t["vl_sb"])
    live = {}

    def stage_a(grp):
        a0 = st.tile([128, GEX, HQ], BF16, tag="a0", bufs=OPTS["abufs"])
        a1 = st.tile([128, GEX, HQ], BF16, tag="a1", bufs=OPTS["abufs"])
        live[grp] = (a0, a1)
        for eg in range(GEX):
            ex = grp * GEX + eg
            l0 = _psum(ps, "t0")[:, 0:HQ]
            l1 = _psum(ps, "t1")[0:K1, 0:HQ]
            nc.tensor.matmul(l0, c["id_sb"], c["bt_sb"][:, 0, :],
                             start=True, stop=False)
            nc.tensor.matmul(l0, k_s[:, ex, 0:K0], qbd[:, ex, :],
                             start=False, stop=True)
            nc.tensor.matmul(l1, c["id_sb"][0:K1, 0:K1], c["bt_sb"][0:K1, 1, :],
                             start=True, stop=False)
            nc.tensor.matmul(l1, k_s[:, ex, K0:N], qbd[:, ex, :],
                             start=False, stop=True)
            nc.scalar.activation(out=a0[:, eg, :], in_=l0, func=AF.Exp)
            nc.scalar.activation(out=a1[0:K1, eg, :], in_=l1, func=AF.Exp)

    def stage_bc(grp):
        a0, a1 = live.pop(grp)
        for eg in range(GEX):
            ex = grp * GEX + eg
            sp = _psum(ps, "t2")[:, 0:HQ]
            nc.tensor.matmul(sp, c["on_sb"], a0[:, eg, :],
                             start=True, stop=False)
            nc.tensor.matmul(sp, c["on_sb"][0:K1, :], a1[0:K1, eg, :],
                             start=False, stop=True)
            rsf = st.tile([128, HQ], F32, tag="rsf")
            nc.vector.reciprocal_approx_fast(out=rsf, in_=sp)
            oe = _psum(ps, "t3")[:, 0:VMC * 2 * N2].rearrange(
                "p (m k q) -> p m k q", m=VMC, k=2, q=N2)
            for m in range(VMC):
                nc.tensor.matmul(oe[:, m, :, :],
                                 vt0[:, ex, 128 * m:128 * (m + 1)],
                                 a0[:, eg, 98 * m:98 * (m + 1)],
                                 start=True, stop=False)
                nc.tensor.matmul(oe[:, m, :, :],
                                 vt1[0:K1, ex, 128 * m:128 * (m + 1)],
                                 a1[0:K1, eg, 98 * m:98 * (m + 1)],
                                 start=False, stop=True)
            rv = rsf.rearrange("p (m k q) -> p m k q", m=VMC, k=2, q=N2)
            nc.vector.tensor_mul(oe[0:64, :, 0, :], oe[0:64, :, 0, :],
                                 rv[0:64, :, 0, :])
            nc.vector.tensor_mul(oe[64:128, :, 1, :], oe[64:128, :, 1, :],
                                 rv[64:128, :, 1, :])
            nc.vector.tensor_add(vl_sb[0:64, :, ex, :],
                                 oe[0:64, :, 0, :],
                                 vl_sb[0:64, :, ex, :])
            nc.vector.tensor_add(vl_sb[64:128, :, ex, :],
                                 oe[64:128, :, 1, :],
                                 vl_sb[64:128, :, ex, :])

    def gelu_half(grp):
        # gelu the 2-group (GEX*2 examples) slice that just finished merging
        e0 = grp * GEX * 2
        ew = GEX * 2
        nc.scalar.activation(out=vl_sb[:, :, e0:e0 + ew, :],
                             in_=vl_sb[:, :, e0:e0 + ew, :], func=AF.Gelu)

    def proj_blk(blk):
        if not parts & 8:
            return
        e0 = blk * NEG
        for m in range(PMC):
            pj = _psum(ps, "t3")[:, 0:NEG * N2].rearrange(
                "p (ee q) -> p ee q", ee=NEG)
            for kc in range(VMC):
                nc.tensor.matmul(pj, c["pw_sb"][:, kc, 128 * m:128 * (m + 1)],
                                 vl_sb[:, kc, e0:e0 + NEG, :],
                                 start=(kc == 0), stop=(kc == VMC - 1))
            ost = st.tile([128, NEG, N2], F32, tag="ost")
            nc.vector.tensor_scalar_add(ost, pj, c["pb_sb"][:, m:m + 1])
            nc.sync.dma_start(
                out=dd["out_d"][e0:e0 + NEG, 128 * m:128 * (m + 1), :]
                .rearrange("e p q -> p e q"),
                in_=ost)

    gpb = NEG // GEX  # groups per block

    def maybe_proj(done_grp):
        if done_grp < 0:
            return
        if (done_grp + 1) % 2 == 0:
            gelu_half((done_grp + 1) // 2 - 1)
        if (done_grp + 1) % gpb == 0:
            proj_blk((done_grp + 1) // gpb - 1)

    OFF = OPTS["off"]  # pipeline depth in groups (stage_a leads stage_bc)
    PD = OPTS["proj_delay"]
    for grp in range(OFF):
        stage_a(grp)
    for grp in range(OFF, ngrp):
        stage_a(grp)
        stage_bc(grp - OFF)
        maybe_proj(grp - OFF - PD)
    for grp in range(ngrp - OFF, ngrp):
        stage_bc(grp)
        maybe_proj(grp - PD)
    for g in range(ngrp - PD, ngrp):
        maybe_proj(g)


_CACHE = {}


def _get_nc(e=E, loops=0, parts=15):
    key = (e, loops, parts)
    if key not in _CACHE:
        _CACHE[key] = build_nc(e, loops, parts)
    return _CACHE[key]


def kernel(**inputs):
    from concourse.bass_utils import run_bass_kernel_spmd

    shared, x_shards = host_prep(inputs)
    nc = _get_nc(E)
    in_maps = [{**shared, **x_shards[c]} for c in range(NCORES)]
    res = run_bass_kernel_spmd(nc, in_maps, core_ids=list(range(NCORES)))
    out = np.concatenate([r["out"] for r in res.results], axis=0)
    return out.reshape(B, OUT_DIM, RES2, RES2).astype(np.float32)
